# revision 16
# baseline (speedup 1.0000x reference)
"""GAT layer (DGL GATConv + ELU + residual) as a Bass/Tile kernel on 8 TRN2 NeuronCores.

Strategy (edge/graph parallelism, dst-sharded):
  - Sort edges by dst on host; shard contiguous dst-node ranges across the 8
    cores (6272 nodes/core = 49 windows of 128). Each core owns all incoming
    edges of its node range, so softmax + scatter-add are core-local and no
    collective is needed.
  - Phase A (replicated on every core): feat = h @ W  -> row-major gather
    table in DRAM ([50176, 256] f32).  Phase A2: er = <feat, attn_r> for the
    core's own nodes (kept SBUF-resident).
  - Phase B per 128-node window: one multi-row indirect DMA gathers
    feat[src[e]] rows for all edges of the window; per 128-edge tile a
    dst-local one-hot matrix (iota + is_equal) both expands er to edges (via
    PE matmul) and scatter-adds exp-weighted messages + softmax denominators
    into a PSUM accumulator ([128 nodes, 260]).  Softmax division is folded
    to node level: rst = U / denom.  Finalize: +bias, ELU, +h residual.
"""

import sys

for p in ("/opt/trn_rl_repo",):
    if p not in sys.path:
        sys.path.insert(0, p)

import numpy as np

import concourse.bass as bass
import concourse.bacc as bacc
import concourse.mybir as mybir
import concourse.tile as tile
from concourse.bass_utils import run_bass_kernel_spmd


F32 = mybir.dt.float32
I32 = mybir.dt.int32
AF = mybir.ActivationFunctionType
OP = mybir.AluOpType

P = 128  # partitions / window size


class Cfg:
    def __init__(self, N=50000, E=800000, H=4, D=64, ncores=8, nwin=49,
                 neg_slope=0.2, split=25088):
        self.N, self.E, self.H, self.D = N, E, H, D
        self.F = H * D
        self.ncores = ncores
        self.nwin = nwin                  # windows (128 nodes) per core
        self.npc = nwin * P               # nodes per core
        self.npad = self.npc * ncores     # padded total nodes
        assert self.npad >= N
        self.neg_slope = neg_slope
        # gather-table split point: dma_gather indices are int16, so the
        # table is gathered in two passes (rows < split / rows >= split)
        self.split = split
        assert split < 32768 and self.npad - split < 32768


def preprocess(cfg, src, dst):
    """Sort edges by dst, assign to (core, window), split each window's
    edges into lo/hi (by src row vs cfg.split), pad each region to a
    multiple of 128, and build the int16 gather-index + dst-local arrays
    in device layout."""
    perm = np.argsort(dst, kind="stable")
    s = src[perm].astype(np.int64)
    d = dst[perm].astype(np.int64)
    gwin = d // P                                   # global window id
    dloc = (d % P).astype(np.float32)
    ngw = cfg.ncores * cfg.nwin
    cnt = np.bincount(gwin, minlength=ngw)
    starts = np.concatenate([[0], np.cumsum(cnt)])
    lo_cnt = np.zeros(ngw, np.int64)
    for g in range(ngw):
        a, b = starts[g], starts[g + 1]
        lo_cnt[g] = int((s[a:b] < cfg.split).sum())
    hi_cnt = cnt - lo_cnt
    t_lo = max(1, int(np.ceil(lo_cnt.max() / P)))
    t_hi = max(1, int(np.ceil(hi_cnt.max() / P)))
    tpw = t_lo + t_hi
    slots = tpw * P
    # per (core,window) slot sequence: [lo edges | pad0][hi edges | pad0]
    idxs = np.zeros((ngw, slots), np.int16)          # pad -> row 0 of region
    dstl = np.full((ngw, slots), 200.0, np.float32)  # pad -> no one-hot match
    for g in range(ngw):
        a, b = starts[g], starts[g + 1]
        sw, dw = s[a:b], dloc[a:b]
        lo = sw < cfg.split
        nl, nh = int(lo.sum()), int((~lo).sum())
        idxs[g, :nl] = sw[lo]
        dstl[g, :nl] = dw[lo]
        hb = t_lo * P
        idxs[g, hb:hb + nh] = sw[~lo] - cfg.split
        dstl[g, hb:hb + nh] = dw[~lo]
    # dma_gather consumes idx j from [j%16, j//16] (16-row block replicated
    # across the 8 groups of 16 partitions); region j is window-local.
    idx_cols = idxs.reshape(cfg.ncores, cfg.nwin * slots)
    idx_dev = np.zeros((cfg.ncores, P, cfg.nwin * slots // 16), np.int16)
    for c in range(cfg.ncores):
        blk = idx_cols[c].reshape(-1, 16).T          # [16, cols]
        idx_dev[c] = np.tile(blk, (8, 1))
    # dstl: gather writes seq pos j -> [j%128, j//128]; tile t of window w is
    # dstl_dev[:, w*tpw + t]
    dstl = dstl.reshape(cfg.ncores, cfg.nwin, tpw, P).transpose(0, 3, 1, 2)
    dstl = np.ascontiguousarray(dstl.reshape(cfg.ncores, P, cfg.nwin * tpw))
    return idx_dev, dstl, t_lo, t_hi


def build(cfg, t_lo, t_hi):
    """Build the SPMD Bass program. Returns nc."""
    N, F, H, D = cfg.npad, cfg.F, cfg.H, cfg.D
    nwin, npc = cfg.nwin, cfg.npc
    tpw = t_lo + t_hi
    KB = F // P            # k blocks (2)
    NBC = 512              # phase-A node chunk
    nchunks = N // NBC if N % NBC == 0 else None
    if nchunks is None:
        NBC = P
        nchunks = N // NBC
    tpn = NBC // P         # node tiles per chunk

    nc = bacc.Bacc("TRN2", target_bir_lowering=False, debug=False,
                   num_devices=cfg.ncores)

    hT = nc.dram_tensor("hT", [F, N], F32, kind="ExternalInput")
    hTo = nc.dram_tensor("hTo", [F, npc], F32, kind="ExternalInput")
    ho = nc.dram_tensor("ho", [npc, F], F32, kind="ExternalInput")
    Wt = nc.dram_tensor("W", [F, F], F32, kind="ExternalInput")
    alr = nc.dram_tensor("alr", [P, F], F32, kind="ExternalInput")
    arr = nc.dram_tensor("arr", [P, F], F32, kind="ExternalInput")
    brep = nc.dram_tensor("brep", [P, F], F32, kind="ExternalInput")
    idx_d = nc.dram_tensor("idx16", [P, nwin * tpw * P // 16],
                           mybir.dt.int16, kind="ExternalInput")
    dstl_d = nc.dram_tensor("dstl", [P, nwin * tpw], F32, kind="ExternalInput")
    iotaf_d = nc.dram_tensor("iotaf", [P, P], F32, kind="ExternalInput")
    iotap_d = nc.dram_tensor("iotap", [P, 1], F32, kind="ExternalInput")
    ident_d = nc.dram_tensor("ident", [P, P], F32, kind="ExternalInput")
    out_d = nc.dram_tensor("out", [npc, F], F32, kind="ExternalOutput")

    with tile.TileContext(nc) as tc:
        with (
            tc.tile_pool(name="const", bufs=1) as cp,
            tc.tile_pool(name="dram", bufs=1, space="DRAM") as dp,
            tc.tile_pool(name="pa", bufs=3) as pa,
            tc.tile_pool(name="paps", bufs=2, space="PSUM") as paps,
            tc.tile_pool(name="pb", bufs=2) as pb,
            tc.tile_pool(name="pbps", bufs=2, space="PSUM") as pbps,
            tc.tile_pool(name="fin", bufs=2) as fin,
        ):
            # ---------------- constants ----------------
            w_sb = cp.tile([P, KB * F], F32)
            for k in range(KB):
                nc.sync.dma_start(w_sb[:, k * F:(k + 1) * F],
                                  Wt[k * P:(k + 1) * P, :])
            alr_sb = cp.tile([P, F], F32)
            nc.sync.dma_start(alr_sb[:], alr[:])
            arr_sb = cp.tile([P, F], F32)
            nc.sync.dma_start(arr_sb[:], arr[:])
            brep_sb = cp.tile([P, F], F32)
            nc.sync.dma_start(brep_sb[:], brep[:])
            idx_sb = cp.tile([P, nwin * tpw * P // 16], mybir.dt.int16)
            nc.sync.dma_start(idx_sb[:], idx_d[:])
            dstl_sb = cp.tile([P, nwin * tpw], F32)
            nc.sync.dma_start(dstl_sb[:], dstl_d[:])

            ident = cp.tile([P, P], F32)
            nc.sync.dma_start(ident[:], ident_d[:])
            iota_f = cp.tile([P, P], F32)
            nc.sync.dma_start(iota_f[:], iotaf_d[:])
            iop_f = cp.tile([P, 1], F32)
            nc.sync.dma_start(iop_f[:], iotap_d[:])

            er_sb = cp.tile([P, nwin * H], F32)
            Tfeat = dp.tile([N, F], F32)

            # ---------------- phase A: gather table ----------------
            for b in range(nchunks):
                hts = []
                for k in range(KB):
                    ht = pa.tile([P, NBC], F32, tag="ht")
                    nc.sync.dma_start(
                        ht[:], hT[k * P:(k + 1) * P, b * NBC:(b + 1) * NBC])
                    hts.append(ht)
                fo = pa.tile([P, tpn * F], F32, tag="fo")
                for i in range(tpn):
                    ps = paps.tile([P, F], F32, tag="pa")
                    for k in range(KB):
                        nc.tensor.matmul(
                            ps[:], lhsT=hts[k][:, i * P:(i + 1) * P],
                            rhs=w_sb[:, k * F:(k + 1) * F],
                            start=(k == 0), stop=(k == KB - 1))
                    nc.scalar.copy(fo[:, i * F:(i + 1) * F], ps[:])
                nc.sync.dma_start(
                    Tfeat[b * NBC:(b + 1) * NBC, :].rearrange(
                        "(i p) f -> p i f", p=P),
                    fo[:].rearrange("p (i f) -> p i f", f=F))

            # ---------------- phase A2: own-range er ----------------
            for w in range(nwin):
                hts = []
                for k in range(KB):
                    ht = pa.tile([P, P], F32, tag="ht2")
                    nc.sync.dma_start(
                        ht[:], hTo[k * P:(k + 1) * P, w * P:(w + 1) * P])
                    hts.append(ht)
                ps = paps.tile([P, F], F32, tag="pa")
                for k in range(KB):
                    nc.tensor.matmul(ps[:], lhsT=hts[k][:],
                                     rhs=w_sb[:, k * F:(k + 1) * F],
                                     start=(k == 0), stop=(k == KB - 1))
                fw = pa.tile([P, F], F32, tag="fw")
                nc.vector.tensor_tensor(out=fw[:], in0=ps[:], in1=arr_sb[:],
                                        op=OP.mult)
                nc.vector.tensor_reduce(
                    out=er_sb[:, w * H:(w + 1) * H],
                    in_=fw[:].rearrange("p (h d) -> p h d", d=D),
                    axis=mybir.AxisListType.X, op=OP.add)

            # ---------------- phase B: edges ----------------
            GMAXT = 8  # max 128-idx tiles per dma_gather (desc-ring capacity)
            for w in range(nwin):
                base = w * tpw
                icol = w * tpw * P // 16   # idx col base for this window
                G = pb.tile([P, tpw * F], F32, tag="G")
                for (rt0, rnt, tb) in [(0, t_lo, Tfeat[0:cfg.split, :]),
                                       (t_lo, t_hi, Tfeat[cfg.split:N, :])]:
                    for c0 in range(rt0, rt0 + rnt, GMAXT):
                        ct = min(GMAXT, rt0 + rnt - c0)
                        nc.gpsimd.dma_gather(
                            out_ap=G[:, c0 * F:(c0 + ct) * F]
                                .rearrange("p (t f) -> p t f", f=F),
                            in_ap=tb,
                            idxs_ap=idx_sb[:, icol + c0 * 8:
                                           icol + (c0 + ct) * 8],
                            num_idxs=ct * P, num_idxs_reg=ct * P,
                            elem_size=F)
                msgx = pb.tile([P, tpw * 264], F32, tag="msgx")
                mx3 = msgx[:].rearrange("p (t x) -> p t x", x=264)
                g4 = G[:].rearrange("p (t h d) -> p t h d", h=H, d=D)

                # el[e,h] = sum_d feat_src[e, h, d] * attn_l[h, d]
                nc.vector.tensor_tensor(
                    out=mx3[:, :, 0:F].rearrange("p t (h d) -> p t h d", d=D),
                    in0=g4,
                    in1=alr_sb[:].rearrange("p (h d) -> p h d", d=D)
                        .unsqueeze(1).to_broadcast([P, tpw, H, D]),
                    op=OP.mult)
                el = pb.tile([P, tpw * H], F32, tag="el")
                nc.vector.tensor_reduce(
                    out=el[:],
                    in_=mx3[:, :, 0:F].rearrange("p t (h d) -> p t h d", d=D),
                    axis=mybir.AxisListType.X, op=OP.add)

                # one-hots + er expansion per tile
                erps = pbps.tile([P, tpw * H], F32, tag="erps")
                oh1 = pb.tile([P, tpw * P], F32, tag="oh1")
                for t in range(tpw):
                    dcol = dstl_sb[:, base + t:base + t + 1]
                    nc.vector.tensor_scalar(
                        out=oh1[:, t * P:(t + 1) * P], in0=iota_f[:],
                        scalar1=dcol, scalar2=None, op0=OP.is_equal)
                    dT = pbps.tile([P, P], F32, tag="dT")
                    nc.tensor.transpose(out=dT[:],
                                        in_=dcol.to_broadcast([P, P]),
                                        identity=ident[:])
                    oh2 = pb.tile([P, P], F32, tag="oh2")
                    nc.vector.tensor_scalar(
                        out=oh2[:], in0=dT[:], scalar1=iop_f[:, :1],
                        scalar2=None, op0=OP.is_equal)
                    nc.tensor.matmul(
                        erps[:, t * H:(t + 1) * H], lhsT=oh2[:],
                        rhs=er_sb[:, w * H:(w + 1) * H],
                        start=True, stop=True)

                # scores -> leaky relu -> exp
                sc = pb.tile([P, tpw * H], F32, tag="sc")
                nc.vector.tensor_tensor(out=sc[:], in0=el[:], in1=erps[:],
                                        op=OP.add)
                sc2 = pb.tile([P, tpw * H], F32, tag="sc2")
                nc.vector.tensor_scalar_mul(sc2[:], sc[:], cfg.neg_slope)
                nc.vector.tensor_tensor(out=sc2[:], in0=sc[:], in1=sc2[:],
                                        op=OP.max)
                ex = pb.tile([P, tpw * H], F32, tag="ex")
                nc.scalar.activation(ex[:], sc2[:], AF.Exp)

                ex3 = ex[:].rearrange("p (t h) -> p t h", h=H)
                # msg = feat_src * ex (broadcast over d), ex appended at col 256
                nc.vector.tensor_tensor(
                    out=mx3[:, :, 0:F].rearrange("p t (h d) -> p t h d", d=D),
                    in0=g4,
                    in1=ex3.unsqueeze(3).to_broadcast([P, tpw, H, D]),
                    op=OP.mult)
                nc.vector.tensor_copy(mx3[:, :, F:F + H], ex3)

                # scatter-add into node accumulator
                acc = pbps.tile([P, F + H], F32, tag="acc")
                for t in range(tpw):
                    nc.tensor.matmul(
                        acc[:], lhsT=oh1[:, t * P:(t + 1) * P],
                        rhs=msgx[:, t * 264:t * 264 + F + H],
                        start=(t == 0), stop=(t == tpw - 1))

                # finalize window
                how = fin.tile([P, F], F32, tag="how")
                nc.sync.dma_start(how[:], ho[w * P:(w + 1) * P, :])
                den = fin.tile([P, H], F32, tag="den")
                nc.vector.tensor_scalar_max(den[:], acc[:, F:F + H], 1e-30)
                rden = fin.tile([P, H], F32, tag="rden")
                nc.vector.reciprocal(rden[:], den[:])
                rst = fin.tile([P, F], F32, tag="rst")
                nc.vector.tensor_tensor(
                    out=rst[:].rearrange("p (h d) -> p h d", d=D),
                    in0=acc[:, 0:F].rearrange("p (h d) -> p h d", d=D),
                    in1=rden[:].unsqueeze(2).to_broadcast([P, H, D]),
                    op=OP.mult)
                nc.vector.tensor_tensor(out=rst[:], in0=rst[:], in1=brep_sb[:],
                                        op=OP.add)
                # ELU: max(x,0)-1 + exp(min(x,0)); then + h residual
                emin = fin.tile([P, F], F32, tag="emin")
                nc.vector.tensor_scalar_min(emin[:], rst[:], 0.0)
                eexp = fin.tile([P, F], F32, tag="eexp")
                nc.scalar.activation(eexp[:], emin[:], AF.Exp)
                erelu = fin.tile([P, F], F32, tag="erelu")
                nc.vector.tensor_scalar(out=erelu[:], in0=rst[:], scalar1=0.0,
                                        scalar2=-1.0, op0=OP.max, op1=OP.add)
                ot = fin.tile([P, F], F32, tag="ot")
                nc.vector.tensor_tensor(out=ot[:], in0=erelu[:], in1=eexp[:],
                                        op=OP.add)
                nc.vector.tensor_tensor(out=ot[:], in0=ot[:], in1=how[:],
                                        op=OP.add)
                nc.sync.dma_start(out_d[w * P:(w + 1) * P, :], ot[:])

    nc.compile()
    return nc


def make_in_maps(cfg, idx_dev, dstl, h, W, attn_l, attn_r, bias):
    hT = np.zeros((cfg.F, cfg.npad), np.float32)
    hT[:, :cfg.N] = np.ascontiguousarray(h.T)
    h_pad = np.zeros((cfg.npad, cfg.F), np.float32)
    h_pad[:cfg.N] = h
    alr = np.tile(attn_l.reshape(1, cfg.F), (P, 1)).astype(np.float32)
    arr = np.tile(attn_r.reshape(1, cfg.F), (P, 1)).astype(np.float32)
    brep = np.tile(bias.reshape(1, cfg.F), (P, 1)).astype(np.float32)
    W = np.ascontiguousarray(W.astype(np.float32))
    iotaf = np.tile(np.arange(P, dtype=np.float32)[None, :], (P, 1))
    iotap = np.arange(P, dtype=np.float32)[:, None].copy()
    ident = np.eye(P, dtype=np.float32)
    in_maps = []
    for c in range(cfg.ncores):
        lo, hi = c * cfg.npc, (c + 1) * cfg.npc
        in_maps.append({
            "hT": hT,
            "hTo": np.ascontiguousarray(hT[:, lo:hi]),
            "ho": np.ascontiguousarray(h_pad[lo:hi]),
            "W": W,
            "alr": alr,
            "arr": arr,
            "brep": brep,
            "idx16": idx_dev[c],
            "dstl": dstl[c],
            "iotaf": iotaf,
            "iotap": iotap,
            "ident": ident,
        })
    return in_maps


_CACHE = {}


def _run(cfg, inputs, **spmd_kwargs):
    h = np.asarray(inputs["h"], np.float32)
    W = np.asarray(inputs["W"], np.float32)
    attn_l = np.asarray(inputs["attn_l"], np.float32)
    attn_r = np.asarray(inputs["attn_r"], np.float32)
    bias = np.asarray(inputs["bias"], np.float32)
    src = np.asarray(inputs["src"])
    dst = np.asarray(inputs["dst"])

    idx_dev, dstl, t_lo, t_hi = preprocess(cfg, src, dst)
    key = (cfg.N, cfg.E, cfg.ncores, cfg.nwin, t_lo, t_hi)
    if key not in _CACHE:
        _CACHE[key] = build(cfg, t_lo, t_hi)
    nc = _CACHE[key]
    in_maps = make_in_maps(cfg, idx_dev, dstl, h, W, attn_l, attn_r, bias)
    res = run_bass_kernel_spmd(nc, in_maps, list(range(cfg.ncores)),
                               **spmd_kwargs)
    outs = [res.results[c]["out"] for c in range(cfg.ncores)]
    full = np.concatenate(outs, axis=0)[:cfg.N]
    return np.ascontiguousarray(full.astype(np.float32)), res


def kernel(h, W, attn_l, attn_r, bias, src, dst):
    cfg = Cfg()
    out, _ = _run(cfg, dict(h=h, W=W, attn_l=attn_l, attn_r=attn_r,
                            bias=bias, src=src, dst=dst))
    return out


def timed_run(cfg, inputs, iters=20):
    """Build the sharded PJRT executable once, keep inputs device-resident,
    and report the min wall-clock of `iters` executions (ns)."""
    import time
    import jax
    import jax.numpy as jnp
    from jax.experimental.shard_map import shard_map
    from jax.sharding import Mesh, NamedSharding, PartitionSpec
    from concourse import bass2jax, mybir as mb

    h = np.asarray(inputs["h"], np.float32)
    W = np.asarray(inputs["W"], np.float32)
    idx_dev, dstl, t_lo, t_hi = preprocess(cfg, np.asarray(inputs["src"]),
                                           np.asarray(inputs["dst"]))
    key = (cfg.N, cfg.E, cfg.ncores, cfg.nwin, t_lo, t_hi)
    if key not in _CACHE:
        _CACHE[key] = build(cfg, t_lo, t_hi)
    nc = _CACHE[key]
    in_maps = make_in_maps(cfg, idx_dev, dstl, h, W,
                           np.asarray(inputs["attn_l"], np.float32),
                           np.asarray(inputs["attn_r"], np.float32),
                           np.asarray(inputs["bias"], np.float32))

    bass2jax.install_neuronx_cc_hook()
    pname = nc.partition_id_tensor.name if nc.partition_id_tensor else None
    in_names, out_names, out_avals, zero_outs = [], [], [], []
    for alloc in nc.m.functions[0].allocations:
        if not isinstance(alloc, mybir.MemoryLocationSet):
            continue
        name = alloc.memorylocations[0].name
        if alloc.kind == "ExternalInput":
            if name != pname:
                in_names.append(name)
        elif alloc.kind == "ExternalOutput":
            shape = tuple(alloc.tensor_shape)
            dtype = mybir.dt.np(alloc.dtype)
            out_names.append(name)
            out_avals.append(jax.core.ShapedArray(shape, dtype))
            zero_outs.append(np.zeros(shape, dtype))
    n_params = len(in_names)
    all_names = in_names + out_names
    if pname is not None:
        all_names = all_names + [pname]

    def _body(*args):
        operands = list(args)
        if pname is not None:
            operands.append(bass2jax.partition_id_tensor())
        outs = bass2jax._bass_exec_p.bind(
            *operands,
            out_avals=tuple(out_avals),
            in_names=tuple(all_names),
            out_names=tuple(out_names),
            lowering_input_output_aliases=(),
            sim_require_finite=True,
            sim_require_nnan=True,
            nc=nc,
        )
        return tuple(outs)

    n = cfg.ncores
    devices = jax.devices()[:n]
    mesh = Mesh(np.asarray(devices), ("core",))
    spec = PartitionSpec("core")
    in_specs = (spec,) * (n_params + len(out_names))
    out_specs = (spec,) * len(out_names)
    fn = jax.jit(shard_map(_body, mesh=mesh, in_specs=in_specs,
                           out_specs=out_specs, check_rep=False),
                 keep_unused=True)
    sh = NamedSharding(mesh, spec)
    args = [
        jax.device_put(
            np.concatenate([np.asarray(in_maps[c][nm]) for c in range(n)],
                           axis=0), sh)
        for nm in in_names
    ] + [
        jax.device_put(np.zeros((n * z.shape[0], *z.shape[1:]), z.dtype), sh)
        for z in zero_outs
    ]
    # warmup
    out = fn(*args)
    jax.block_until_ready(out)
    best = float("inf")
    for _ in range(iters):
        t0 = time.perf_counter()
        out = fn(*args)
        jax.block_until_ready(out)
        best = min(best, time.perf_counter() - t0)
    # pipelined: queue a batch, block once — amortizes host->worker dispatch
    t0 = time.perf_counter()
    outs = [fn(*args) for _ in range(iters)]
    jax.block_until_ready(outs)
    piped = (time.perf_counter() - t0) / iters
    return min(best, piped) * 1e9
